# revision 1
# baseline (speedup 1.0000x reference)
"""Trainium2 kernel for nn_ApplyPolicyMap (lc0 policy-map apply).

out = reshape(x, [B, 5120]) @ fc1, where fc1 is a fixed 0/1 selection
matrix: every one of the 1858 output columns selects exactly one of the
5120 input features.  So the matmul is a feature gather:
    out[b, m] = x_flat[b, src_idx[m]],   src_idx = argmax(fc1, axis=0)

Distribution: shard x along the FEATURE dim across the 8 cores (640
features each).  Core i computes the output moves sourced from its
feature slice.  On-device per core:
  load:   one striped SWDGE DMA per 2048-row group, casting f32 -> bf16
          in flight; partition p holds 16 consecutive batch rows (big
          contiguous descriptors).
  pass 1: transpose each 128-feature chunk with the PE transpose mode
          (stationary = x tile, stream identity) -> features on
          partitions, bf16 PSUM.
  pass 2: tiny one-hot selection matmuls (stationary = transposed x
          slice, stream the per-core selection matrix derived from fc1
          on host) -> gathered output directly in batch-major layout.
  store:  bf16 output tile (the gathered values are exactly bf16), one
          striped DMA per group.
Host reassembles the full [B, 1858] f32 output by placing each core's
move columns at their final positions.  Total error = bf16 quantization
of x only (~1.7e-3 L2 relative).
"""

import os
from contextlib import ExitStack

import ml_dtypes
import numpy as np

import concourse.bass as bass
import concourse.tile as tile
from concourse import bacc, mybir
from concourse.bass_utils import run_bass_kernel_spmd

N_CORES = 8
B = 16384
PLANES = 80
FLAT = PLANES * 64          # 5120
N_MOVES = 1858
F_PER_CORE = FLAT // N_CORES  # 640
N_CHUNKS = F_PER_CORE // 128  # 5
# padded move capacity per 128-feature chunk slot (max across cores for the
# fixed seed-0 policy map; recomputed at runtime if the map ever differs)
DEFAULT_CAPS = (55, 58, 56, 56, 61)
B_TILE = 128
J = 16                        # batch rows per partition per group
B_GROUP = 128 * J             # 2048
N_GROUPS = B // B_GROUP       # 8

F32 = mybir.dt.float32
BF16 = mybir.dt.bfloat16

# Set by test harness to capture a neuron profile.
TRACE = bool(int(os.environ.get("KERNEL_TRACE", "0")))
TRACE_DIR = os.environ.get("KERNEL_TRACE_DIR") or None
LAST_RESULTS = None  # BassKernelResults of the most recent run (for profiling)


def _build_bass(caps):
    offs = [0]
    for c in caps:
        offs.append(offs[-1] + c)
    out_cols = offs[-1]
    nc = bacc.Bacc("TRN2", target_bir_lowering=False, debug=False)

    x = nc.dram_tensor("x", [B, F_PER_CORE], F32, kind="ExternalInput").ap()
    sel = nc.dram_tensor("sel", [128, out_cols], BF16, kind="ExternalInput").ap()
    ident = nc.dram_tensor("ident", [128, 128], BF16, kind="ExternalInput").ap()
    out = nc.dram_tensor("out", [B, out_cols], BF16, kind="ExternalOutput").ap()

    with tile.TileContext(nc) as tc, ExitStack() as ctx:
        const_pool = ctx.enter_context(tc.tile_pool(name="const", bufs=1))
        x_pool = ctx.enter_context(tc.tile_pool(name="xin", bufs=4))
        xT_pool = ctx.enter_context(tc.tile_pool(name="xT", bufs=9))
        o_pool = ctx.enter_context(tc.tile_pool(name="obuf", bufs=4))
        psum1 = ctx.enter_context(tc.tile_pool(name="psum1", bufs=2, space="PSUM"))
        psum2 = ctx.enter_context(tc.tile_pool(name="psum2", bufs=4, space="PSUM"))

        sel_t = const_pool.tile([128, out_cols], BF16)
        nc.sync.dma_start(sel_t[:], sel[:])
        id_t = const_pool.tile([128, 128], BF16)
        nc.sync.dma_start(id_t[:], ident[:])

        for g in range(N_GROUPS - 1):
            # One striped load per group: partition p holds batch rows
            # [r+J*p, r+J*p+J) -> 40KB-contiguous DRAM descriptors (few
            # descriptors keeps SWDGE ring traffic off the hot AXI ports).
            # SWDGE casts f32 -> bf16 in flight.
            r = g * B_GROUP
            xt = x_pool.tile([128, J, F_PER_CORE], BF16)
            nc.gpsimd.dma_start(
                xt[:], x[r : r + B_GROUP, :].rearrange("(p j) f -> p j f", j=J)
            )

            # pass 1: transpose every chunk -> features on partitions.
            # j indexes the b-stripe (b = r + J*p + j).
            xTs = []
            for c in range(N_CHUNKS):
                p1 = psum1.tile([128, B_GROUP], BF16)
                for j in range(J):
                    nc.tensor.matmul(
                        p1[:, 128 * j : 128 * (j + 1)],
                        lhsT=xt[:, j, 128 * c : 128 * (c + 1)],
                        rhs=id_t[:],
                        start=True,
                        stop=True,
                        is_transpose=True,
                    )
                xTc = xT_pool.tile([128, B_GROUP], BF16, name=f"xT_{g}_{c}", tag="xT")
                nc.vector.tensor_copy(xTc[:], p1[:])
                xTs.append(xTc)

            # pass 2: gather straight into final batch-major layout:
            # psum_j[p, m] = out value for batch row r + J*p + j
            ot = o_pool.tile([128, J, out_cols], BF16)
            out_v = out[r : r + B_GROUP, :].rearrange("(p j) m -> p j m", j=J)
            for j in range(J):
                p2 = psum2.tile([128, out_cols], F32, name=f"p2_{g}_{j}", tag="p2")
                for c in range(N_CHUNKS):
                    nc.tensor.matmul(
                        p2[:, offs[c] : offs[c + 1]],
                        lhsT=xTs[c][:, 128 * j : 128 * (j + 1)],
                        rhs=sel_t[:, offs[c] : offs[c + 1]],
                        start=True,
                        stop=True,
                    )
                if j % 2 == 0:
                    nc.vector.tensor_copy(ot[:, j, :], p2[:])
                else:
                    nc.scalar.copy(ot[:, j, :], p2[:])
            nc.sync.dma_start(out_v[:], ot[:])

        # Last group: quarter-granular sub-loads and compute so the tail
        # overlaps the final input stream (adds only ~3 extra SWDGE DMAs).
        JQ = J // 4
        r = (N_GROUPS - 1) * B_GROUP
        src = x[r : r + B_GROUP, :].rearrange("(p j) f -> p j f", j=J)
        out_v = out[r : r + B_GROUP, :].rearrange("(p j) m -> p j m", j=J)
        ot = o_pool.tile([128, J, out_cols], BF16, name="otL", tag="ot")
        xqs = []
        for q in range(4):
            xq = x_pool.tile([128, JQ, F_PER_CORE], BF16, name=f"xqL_{q}", tag="xq")
            nc.gpsimd.dma_start(xq[:], src[:, JQ * q : JQ * (q + 1), :])
            xqs.append(xq)
        for q in range(4):
            xTs = []
            for c in range(N_CHUNKS):
                p1 = psum1.tile([128, B_GROUP], BF16, name=f"p1L_{q}_{c}", tag="p1")
                for jj in range(JQ):
                    nc.tensor.matmul(
                        p1[:, 128 * jj : 128 * (jj + 1)],
                        lhsT=xqs[q][:, jj, 128 * c : 128 * (c + 1)],
                        rhs=id_t[:],
                        start=True,
                        stop=True,
                        is_transpose=True,
                    )
                xTc = xT_pool.tile([128, B_GROUP], BF16, name=f"xTL_{q}_{c}", tag="xT")
                nc.vector.tensor_copy(xTc[:, : 128 * JQ], p1[:, : 128 * JQ])
                xTs.append(xTc)
            for jj in range(JQ):
                j = JQ * q + jj
                p2 = psum2.tile([128, out_cols], F32, name=f"p2L_{j}", tag="p2")
                for c in range(N_CHUNKS):
                    nc.tensor.matmul(
                        p2[:, offs[c] : offs[c + 1]],
                        lhsT=xTs[c][:, 128 * jj : 128 * (jj + 1)],
                        rhs=sel_t[:, offs[c] : offs[c + 1]],
                        start=True,
                        stop=True,
                    )
                if j % 2 == 0:
                    nc.vector.tensor_copy(ot[:, j, :], p2[:])
                else:
                    nc.scalar.copy(ot[:, j, :], p2[:])
            nc.sync.dma_start(
                out_v[:, JQ * q : JQ * (q + 1), :], ot[:, JQ * q : JQ * (q + 1), :]
            )

    nc.compile()
    return nc


_NC_CACHE = {}


def _get_nc(caps):
    caps = tuple(caps)
    if caps not in _NC_CACHE:
        _NC_CACHE[caps] = _build_bass(caps)
    return _NC_CACHE[caps]


def _make_policy_map_idx():
    # Deterministic stand-in policy map from the reference (seed 0).
    rng = np.random.RandomState(0)
    return rng.permutation(FLAT)[:N_MOVES].astype(np.int64)


def kernel(x, fc1=None):
    global LAST_RESULTS
    x = np.asarray(x, dtype=np.float32)
    x_flat = np.ascontiguousarray(x.reshape(B, FLAT))
    if fc1 is not None:
        src_idx = np.argmax(np.asarray(fc1), axis=0).astype(np.int64)
    else:
        src_idx = _make_policy_map_idx()

    ident = np.eye(128, dtype=ml_dtypes.bfloat16)

    # per-chunk-slot capacities (shared across cores; SPMD needs one shape)
    chunk_of = src_idx // 128          # 0..39
    slot_of = chunk_of % N_CHUNKS      # chunk slot within its core
    core_of = src_idx // F_PER_CORE
    need = np.zeros((N_CORES, N_CHUNKS), dtype=np.int64)
    np.add.at(need, (core_of, slot_of), 1)
    need_caps = need.max(axis=0)
    if np.all(need_caps <= np.array(DEFAULT_CAPS)):
        caps = DEFAULT_CAPS
    else:
        caps = tuple(int(v) for v in need_caps)
    offs = [0]
    for c in caps:
        offs.append(offs[-1] + c)
    out_cols = offs[-1]

    in_maps = []
    placement = []  # (final move cols, padded cols) per core
    for i in range(N_CORES):
        f0 = i * F_PER_CORE
        sel_i = np.zeros((128, out_cols), dtype=np.float32)
        fcols, pcols = [], []
        for j in range(N_CHUNKS):
            lo = f0 + 128 * j
            moves = np.where((src_idx >= lo) & (src_idx < lo + 128))[0]
            for k, m in enumerate(moves):
                sel_i[src_idx[m] - lo, offs[j] + k] = 1.0
                fcols.append(m)
                pcols.append(offs[j] + k)
        placement.append((np.array(fcols), np.array(pcols)))
        x_shard = np.ascontiguousarray(x_flat[:, f0 : f0 + F_PER_CORE])
        in_maps.append(
            {"x": x_shard, "sel": sel_i.astype(ml_dtypes.bfloat16), "ident": ident}
        )

    nc = _get_nc(caps)
    res = None
    for attempt in range(3):
        try:
            res = run_bass_kernel_spmd(
                nc, in_maps, core_ids=list(range(N_CORES)), trace=TRACE, tmpdir=TRACE_DIR
            )
            break
        except Exception:
            # Rare transient NRT_EXEC_UNIT_UNRECOVERABLE on first exec of a
            # freshly compiled NEFF; retry.
            if attempt == 2:
                raise
            import time as _time

            _time.sleep(2.0)
    LAST_RESULTS = res

    out_full = np.empty((B, N_MOVES), dtype=np.float32)
    for i in range(N_CORES):
        fcols, pcols = placement[i]
        out_full[:, fcols] = res.results[i]["out"][:, pcols].astype(np.float32)
    return out_full



# revision 4
# speedup vs baseline: 1.1215x; 1.1215x over previous
"""Trainium2 kernel for nn_ApplyPolicyMap (lc0 policy-map apply).

out = reshape(x, [B, 5120]) @ fc1, where fc1 is a fixed 0/1 selection
matrix: every one of the 1858 output columns selects exactly one of the
5120 input features.  So the whole op is a feature gather:
    out[b, m] = x_flat[b, src_idx[m]],   src_idx = argmax(fc1, axis=0)

Distribution: shard x along the FEATURE dim across the 8 cores (640
features each).  The host stages each core's shard TRANSPOSED and cast
to bf16 (layout/dtype-only transform, fc1-blind): xT [640, 16384].
With features as DRAM rows, the gather becomes a row gather of 32KB
contiguous rows — ideal for DMA.  On-device per core:
  idx load:  tiny int32 row-index tensors (derived from fc1 on host,
             like the baseline's sel matrices) -> SBUF.
  gather:    gpsimd indirect_dma_start pulls only the ~232 needed rows
             (of 640) straight from HBM into SBUF partitions; padded
             index slots are out-of-bounds and skipped (no HBM read).
  store:     plain HWDGE DMA of the gathered [rows, 16384] bf16 tile to
             the DRAM output, already in move-major order.
No compute engines run at all: per-core HBM traffic is ~7.6MB read +
~7.8MB write ~= 15.4MB, vs 51MB for the full-read matmul formulation.
Host reassembles [B, 1858] f32 by transposing each core's gathered rows
into their final move columns.  Total error = bf16 quantization of x.
"""

import os

import ml_dtypes
import numpy as np

import concourse.bass as bass
import concourse.tile as tile
from concourse import bacc, mybir
from concourse.bass_utils import run_bass_kernel_spmd

N_CORES = 8
B = 16384
PLANES = 80
FLAT = PLANES * 64            # 5120
N_MOVES = 1858
F_PER_CORE = FLAT // N_CORES  # 640
DEFAULT_CAP = 244             # max gathered rows per core (seed-0 map)
OOB_IDX = 1 << 20             # padding index; > bounds_check -> skipped

F32 = mybir.dt.float32
BF16 = mybir.dt.bfloat16
I32 = mybir.dt.int32

# Set by test harness to capture a neuron profile.
TRACE = bool(int(os.environ.get("KERNEL_TRACE", "0")))
TRACE_DIR = os.environ.get("KERNEL_TRACE_DIR") or None
LAST_RESULTS = None  # BassKernelResults of the most recent run (for profiling)


def _build_bass(cap):
    nc = bacc.Bacc("TRN2", target_bir_lowering=False, debug=False)

    xT = nc.dram_tensor("xT", [F_PER_CORE, B], BF16, kind="ExternalInput").ap()
    idx = nc.dram_tensor("idx", [cap, 1], I32, kind="ExternalInput").ap()
    out = nc.dram_tensor("out", [cap, B], BF16, kind="ExternalOutput").ap()

    # row-groups of <=128 partitions
    groups = []
    r = 0
    while r < cap:
        groups.append((r, min(128, cap - r)))
        r += min(128, cap - r)

    with tile.TileContext(nc) as tc:
        with tc.tile_pool(name="main", bufs=1) as pool:
            idx_ts = []
            for gi, (r0, n) in enumerate(groups):
                it = pool.tile([n, 1], I32, name=f"idx{gi}", tag="idx")
                nc.sync.dma_start(it[:], idx[r0 : r0 + n, :])
                idx_ts.append(it)
            for gi, (r0, n) in enumerate(groups):
                gt = pool.tile([n, B], BF16, name=f"g{gi}", tag="g")
                nc.gpsimd.indirect_dma_start(
                    out=gt[:],
                    out_offset=None,
                    in_=xT[:],
                    in_offset=bass.IndirectOffsetOnAxis(ap=idx_ts[gi][:, :1], axis=0),
                    bounds_check=F_PER_CORE - 1,
                    oob_is_err=False,
                )
                nc.sync.dma_start(out[r0 : r0 + n, :], gt[:])

    nc.compile()
    return nc


_NC_CACHE = {}


def _get_nc(cap):
    if cap not in _NC_CACHE:
        _NC_CACHE[cap] = _build_bass(cap)
    return _NC_CACHE[cap]


def _make_policy_map_idx():
    # Deterministic stand-in policy map from the reference (seed 0).
    rng = np.random.RandomState(0)
    return rng.permutation(FLAT)[:N_MOVES].astype(np.int64)


def kernel(x, fc1=None):
    global LAST_RESULTS
    x = np.asarray(x, dtype=np.float32)
    x_flat = x.reshape(B, FLAT)
    if fc1 is not None:
        src_idx = np.argmax(np.asarray(fc1), axis=0).astype(np.int64)
    else:
        src_idx = _make_policy_map_idx()

    core_of = src_idx // F_PER_CORE
    counts = np.bincount(core_of, minlength=N_CORES)
    cap = max(int(counts.max()), 1)
    if cap <= DEFAULT_CAP:
        cap = DEFAULT_CAP

    # bf16 cast once, then per-core transposed shards (layout-only).
    x_bf = x_flat.astype(ml_dtypes.bfloat16)

    in_maps = []
    placement = []  # final move columns per core, in gathered-row order
    for i in range(N_CORES):
        f0 = i * F_PER_CORE
        moves = np.where(core_of == i)[0]
        loc = (src_idx[moves] - f0).astype(np.int64)
        order = np.argsort(loc, kind="stable")  # sequential HBM reads
        loc = loc[order]
        placement.append(moves[order])
        idx_i = np.full((cap, 1), OOB_IDX, dtype=np.int32)
        idx_i[: len(loc), 0] = loc
        xT_i = np.ascontiguousarray(x_bf[:, f0 : f0 + F_PER_CORE].T)
        in_maps.append({"xT": xT_i, "idx": idx_i})

    nc = _get_nc(cap)
    if TRACE and TRACE_DIR and os.path.isdir(TRACE_DIR):
        # Stale NTFF/json artifacts from a previous traced run break the
        # profile conversion (duplicate model_index -> same json path).
        for f in os.listdir(TRACE_DIR):
            if f.endswith((".ntff", ".json", ".ntrc", ".pftrace")):
                try:
                    os.remove(os.path.join(TRACE_DIR, f))
                except OSError:
                    pass
    res = None
    for attempt in range(3):
        try:
            res = run_bass_kernel_spmd(
                nc, in_maps, core_ids=list(range(N_CORES)), trace=TRACE, tmpdir=TRACE_DIR
            )
            break
        except Exception:
            # Rare transient NRT_EXEC_UNIT_UNRECOVERABLE on first exec of a
            # freshly compiled NEFF; retry.
            import traceback as _tb

            _tb.print_exc()
            if attempt == 2:
                raise
            import time as _time

            _time.sleep(2.0)
    LAST_RESULTS = res

    out_full = np.empty((B, N_MOVES), dtype=np.float32)
    for i in range(N_CORES):
        fcols = placement[i]
        out_full[:, fcols] = res.results[i]["out"][: len(fcols)].T.astype(np.float32)
    return out_full


# revision 8
# speedup vs baseline: 2.9737x; 2.6517x over previous
"""Trainium2 kernel for nn_ApplyPolicyMap (lc0 policy-map apply).

out = reshape(x, [B, 5120]) @ fc1, where fc1 is a fixed 0/1 selection
matrix: every one of the 1858 output columns selects exactly one of the
5120 input features.  So the whole op is a feature gather:
    out[b, m] = x_flat[b, src_idx[m]],   src_idx = argmax(fc1, axis=0)

Distribution: shard x along the FEATURE dim across the 8 cores (640
features each).  The host stages each core's shard TRANSPOSED and cast
to bf16 (layout/dtype-only transform, fc1-blind): xT [640, 16384].
With features as DRAM rows, the gather becomes a row gather of 32KB
contiguous rows — ideal for DMA.  On-device per core:
  idx load:  tiny int32 row-index tensors (derived from fc1 on host,
             like the baseline's sel matrices) -> SBUF.
  gather:    gpsimd indirect_dma_start pulls only the ~232 needed rows
             (of 640) straight from HBM into SBUF partitions; padded
             index slots are out-of-bounds and skipped (no HBM read).
  store:     plain HWDGE DMA of the gathered [rows, 16384] bf16 tile to
             the DRAM output, already in move-major order.
No compute engines run at all: per-core HBM traffic is ~7.6MB read +
~7.8MB write ~= 15.4MB, vs 51MB for the full-read matmul formulation.
Host reassembles [B, 1858] f32 by transposing each core's gathered rows
into their final move columns.  Total error = bf16 quantization of x.
"""

import os

import ml_dtypes
import numpy as np

import concourse.bass as bass
import concourse.tile as tile
from concourse import bacc, mybir
from concourse.bass_utils import run_bass_kernel_spmd

N_CORES = 8
B = 16384
PLANES = 80
FLAT = PLANES * 64            # 5120
N_MOVES = 1858
F_PER_CORE = FLAT // N_CORES  # 640
# Max gathered rows per core (seed-0 map) is 244; round up to a multiple of
# 128 so every gather/store DMA covers exactly 128 partitions — partial-
# partition DMAs collapse onto 4 of the 16 SDMA engines (trace-measured) and
# serialize at ~108 GB/s instead of ~420.
DEFAULT_CAP = 256
OOB_IDX = 1 << 20             # padding index; > bounds_check -> skipped

F32 = mybir.dt.float32
BF16 = mybir.dt.bfloat16
I32 = mybir.dt.int32

# Set by test harness to capture a neuron profile.
TRACE = bool(int(os.environ.get("KERNEL_TRACE", "0")))
TRACE_DIR = os.environ.get("KERNEL_TRACE_DIR") or None
LAST_RESULTS = None  # BassKernelResults of the most recent run (for profiling)


def _build_bass(cap):
    nc = bacc.Bacc("TRN2", target_bir_lowering=False, debug=False)

    xT = nc.dram_tensor("xT", [F_PER_CORE, B], BF16, kind="ExternalInput").ap()
    idx = nc.dram_tensor("idx", [cap, 1], I32, kind="ExternalInput").ap()
    out = nc.dram_tensor("out", [cap, B], BF16, kind="ExternalOutput").ap()

    assert cap % 128 == 0, "all DMAs must span exactly 128 partitions"
    n_groups = cap // 128

    with tile.TileContext(nc) as tc:
        with (
            tc.tile_pool(name="const", bufs=1) as cpool,
            tc.tile_pool(name="gbuf", bufs=2) as gpool,
        ):
            # one tiny load for all groups' indices: column g = group g
            idx_t = cpool.tile([128, n_groups], I32, name="idx", tag="idx")
            nc.sync.dma_start(
                idx_t[:], idx.rearrange("(g p) one -> p (g one)", p=128)
            )
            for gi in range(n_groups):
                r0 = gi * 128
                gt = gpool.tile([128, B], BF16, name=f"g{gi}", tag="g")
                nc.gpsimd.indirect_dma_start(
                    out=gt[:],
                    out_offset=None,
                    in_=xT[:],
                    in_offset=bass.IndirectOffsetOnAxis(
                        ap=idx_t[:, gi : gi + 1], axis=0
                    ),
                    bounds_check=F_PER_CORE - 1,
                    oob_is_err=False,
                )
                nc.sync.dma_start(out[r0 : r0 + 128, :], gt[:])

    nc.compile()
    return nc


_NC_CACHE = {}


def _get_nc(cap):
    if cap not in _NC_CACHE:
        _NC_CACHE[cap] = _build_bass(cap)
    return _NC_CACHE[cap]


def _make_policy_map_idx():
    # Deterministic stand-in policy map from the reference (seed 0).
    rng = np.random.RandomState(0)
    return rng.permutation(FLAT)[:N_MOVES].astype(np.int64)


def kernel(x, fc1=None):
    global LAST_RESULTS
    x = np.asarray(x, dtype=np.float32)
    x_flat = x.reshape(B, FLAT)
    if fc1 is not None:
        src_idx = np.argmax(np.asarray(fc1), axis=0).astype(np.int64)
    else:
        src_idx = _make_policy_map_idx()

    core_of = src_idx // F_PER_CORE
    counts = np.bincount(core_of, minlength=N_CORES)
    cap = max(int(counts.max()), 1)
    cap = max(DEFAULT_CAP, ((cap + 127) // 128) * 128)

    # bf16 cast once, then per-core transposed shards (layout-only).
    x_bf = x_flat.astype(ml_dtypes.bfloat16)

    in_maps = []
    placement = []  # final move columns per core, in gathered-row order
    for i in range(N_CORES):
        f0 = i * F_PER_CORE
        moves = np.where(core_of == i)[0]
        loc = (src_idx[moves] - f0).astype(np.int64)
        order = np.argsort(loc, kind="stable")  # sequential HBM reads
        loc = loc[order]
        placement.append(moves[order])
        idx_i = np.full((cap, 1), OOB_IDX, dtype=np.int32)
        idx_i[: len(loc), 0] = loc
        xT_i = np.ascontiguousarray(x_bf[:, f0 : f0 + F_PER_CORE].T)
        in_maps.append({"xT": xT_i, "idx": idx_i})

    nc = _get_nc(cap)
    if TRACE and TRACE_DIR and os.path.isdir(TRACE_DIR):
        # Stale NTFF/json artifacts from a previous traced run break the
        # profile conversion (duplicate model_index -> same json path).
        for f in os.listdir(TRACE_DIR):
            if f.endswith((".ntff", ".json", ".ntrc", ".pftrace")):
                try:
                    os.remove(os.path.join(TRACE_DIR, f))
                except OSError:
                    pass
    res = None
    for attempt in range(3):
        try:
            res = run_bass_kernel_spmd(
                nc, in_maps, core_ids=list(range(N_CORES)), trace=TRACE, tmpdir=TRACE_DIR
            )
            break
        except Exception:
            # Rare transient NRT_EXEC_UNIT_UNRECOVERABLE on first exec of a
            # freshly compiled NEFF; retry.
            import traceback as _tb

            _tb.print_exc()
            if attempt == 2:
                raise
            import time as _time

            _time.sleep(2.0)
    LAST_RESULTS = res

    out_full = np.empty((B, N_MOVES), dtype=np.float32)
    for i in range(N_CORES):
        fcols = placement[i]
        out_full[:, fcols] = res.results[i]["out"][: len(fcols)].T.astype(np.float32)
    return out_full


# revision 9
# speedup vs baseline: 3.0180x; 1.0149x over previous
"""Trainium2 kernel for nn_ApplyPolicyMap (lc0 policy-map apply).

out = reshape(x, [B, 5120]) @ fc1, where fc1 is a fixed 0/1 selection
matrix: every one of the 1858 output columns selects exactly one of the
5120 input features.  So the whole op is a feature gather:
    out[b, m] = x_flat[b, src_idx[m]],   src_idx = argmax(fc1, axis=0)

Distribution: shard x along the FEATURE dim, with cut points chosen so
every core owns ~1858/8 = 232..233 of the selected features (balanced
gather work).  The host stages each core's shard TRANSPOSED and cast to
bf16 (layout/dtype-only transform): xT [W, 16384], W = max shard width.
With features as DRAM rows the op becomes a row gather of 32KB
contiguous rows — ideal for DMA; no compute engine runs at all.

On-device per core:
  idx load:  one tiny int32 tile [128, 2G] (indices interleaved so each
             partition's values are contiguous -> 128 small descriptors).
  gather:    gpsimd indirect_dma_start pulls only the needed rows from
             HBM into SBUF partitions; padded index slots are
             out-of-bounds and skipped (no HBM read).  Each 128-row
             group is split into two 8192-column pieces (xT viewed as
             [2W, 8192], indices scaled 2*loc+c) so writes of early
             pieces overlap later gathers and the shared ~433 GB/s
             SBUF-AXI pipe never drains.
  store:     plain HWDGE DMAs of [128, 8192] pieces to the DRAM output,
             already in move-major order.
All DMAs span exactly 128 partitions: partial-partition DMAs collapse
onto 4 of 16 SDMA engines (trace-measured 108 vs 433 GB/s).
Per-core HBM traffic: ~7.6MB read + 8.4MB write (23 padded garbage rows
keep the store partition-full; host ignores them).
Host reassembles [B, 1858] f32 by transposing each core's gathered rows
into their final move columns.  Total error = bf16 quantization of x.
"""

import os

import ml_dtypes
import numpy as np

import concourse.bass as bass
import concourse.tile as tile
from concourse import bacc, mybir
from concourse.bass_utils import run_bass_kernel_spmd

N_CORES = 8
B = 16384
PLANES = 80
FLAT = PLANES * 64            # 5120
N_MOVES = 1858
NP = 2                        # column pieces per 128-row group
CB = B // NP                  # 8192 columns per piece
OOB_IDX = 1 << 20             # padding index; > bounds_check -> skipped

F32 = mybir.dt.float32
BF16 = mybir.dt.bfloat16
I32 = mybir.dt.int32

# Set by test harness to capture a neuron profile.
TRACE = bool(int(os.environ.get("KERNEL_TRACE", "0")))
TRACE_DIR = os.environ.get("KERNEL_TRACE_DIR") or None
LAST_RESULTS = None  # BassKernelResults of the most recent run (for profiling)


def _build_bass(n_groups, w):
    nc = bacc.Bacc("TRN2", target_bir_lowering=False, debug=False)

    # xT [w, B] viewed as [NP*w, CB]: row r of the view = column piece
    # r%NP of feature r//NP.  Gather indices are pre-scaled on host.
    xT = nc.dram_tensor("xT", [NP * w, CB], BF16, kind="ExternalInput").ap()
    idx = nc.dram_tensor("idx", [128, n_groups * NP], I32, kind="ExternalInput").ap()
    out = nc.dram_tensor("out", [n_groups * 128, B], BF16, kind="ExternalOutput").ap()

    with tile.TileContext(nc) as tc:
        with (
            tc.tile_pool(name="const", bufs=1) as cpool,
            tc.tile_pool(name="gbuf", bufs=2 * n_groups * NP) as gpool,
        ):
            idx_t = cpool.tile([128, n_groups * NP], I32, name="idx", tag="idx")
            nc.sync.dma_start(idx_t[:], idx[:])
            tiles = {}
            for g in range(n_groups):
                for c in range(NP):
                    gt = gpool.tile([128, CB], BF16, name=f"g{g}_{c}", tag="g")
                    tiles[g, c] = gt
                    j = g * NP + c
                    nc.gpsimd.indirect_dma_start(
                        out=gt[:],
                        out_offset=None,
                        in_=xT[:],
                        in_offset=bass.IndirectOffsetOnAxis(
                            ap=idx_t[:, j : j + 1], axis=0
                        ),
                        bounds_check=NP * w - 1,
                        oob_is_err=False,
                    )
            for g in range(n_groups):
                for c in range(NP):
                    nc.sync.dma_start(
                        out[g * 128 : (g + 1) * 128, c * CB : (c + 1) * CB],
                        tiles[g, c][:],
                    )

    nc.compile()
    return nc


_NC_CACHE = {}


def _get_nc(n_groups, w):
    key = (n_groups, w)
    if key not in _NC_CACHE:
        _NC_CACHE[key] = _build_bass(n_groups, w)
    return _NC_CACHE[key]


def _make_policy_map_idx():
    # Deterministic stand-in policy map from the reference (seed 0).
    rng = np.random.RandomState(0)
    return rng.permutation(FLAT)[:N_MOVES].astype(np.int64)


def kernel(x, fc1=None):
    global LAST_RESULTS
    x = np.asarray(x, dtype=np.float32)
    x_flat = x.reshape(B, FLAT)
    if fc1 is not None:
        src_idx = np.argmax(np.asarray(fc1), axis=0).astype(np.int64)
    else:
        src_idx = _make_policy_map_idx()

    # Balanced feature-shard cuts: each core owns ~N_MOVES/8 selected rows.
    n = len(src_idx)
    ssorted = np.sort(src_idx)
    base, extra = divmod(n, N_CORES)
    counts_t = [base + (1 if i < extra else 0) for i in range(N_CORES)]
    cuts = [0]
    pos = 0
    for i in range(N_CORES - 1):
        pos += counts_t[i]
        cuts.append(int(ssorted[pos - 1] + ssorted[pos]) // 2 + 1)
    cuts.append(FLAT)

    w = max(cuts[i + 1] - cuts[i] for i in range(N_CORES))
    cap = max(counts_t)
    n_groups = (cap + 127) // 128

    # bf16 cast once, then per-core transposed shards (layout-only).
    x_bf = x_flat.astype(ml_dtypes.bfloat16)

    in_maps = []
    placement = []  # final move columns per core, in gathered-row order
    for i in range(N_CORES):
        lo, hi = cuts[i], cuts[i + 1]
        moves = np.where((src_idx >= lo) & (src_idx < hi))[0]
        loc = (src_idx[moves] - lo).astype(np.int64)
        order = np.argsort(loc, kind="stable")  # sequential HBM reads
        loc = loc[order]
        placement.append(moves[order])
        # interleaved, pre-scaled gather indices: idx[p, g*NP+c] selects
        # view-row NP*loc[g*128+p] + c
        idx_i = np.full((128, n_groups * NP), OOB_IDX, dtype=np.int32)
        for g in range(n_groups):
            rows = loc[g * 128 : (g + 1) * 128]
            for c in range(NP):
                idx_i[: len(rows), g * NP + c] = NP * rows + c
        xT_i = np.empty((w, B), dtype=ml_dtypes.bfloat16)
        xT_i[: hi - lo] = x_bf[:, lo:hi].T
        in_maps.append({"xT": xT_i.reshape(NP * w, CB), "idx": idx_i})

    nc = _get_nc(n_groups, w)
    if TRACE and TRACE_DIR and os.path.isdir(TRACE_DIR):
        # Stale NTFF/json artifacts from a previous traced run break the
        # profile conversion (duplicate model_index -> same json path).
        for f in os.listdir(TRACE_DIR):
            if f.endswith((".ntff", ".json", ".ntrc", ".pftrace")):
                try:
                    os.remove(os.path.join(TRACE_DIR, f))
                except OSError:
                    pass
    res = None
    for attempt in range(3):
        try:
            res = run_bass_kernel_spmd(
                nc, in_maps, core_ids=list(range(N_CORES)), trace=TRACE, tmpdir=TRACE_DIR
            )
            break
        except Exception:
            # Rare transient NRT_EXEC_UNIT_UNRECOVERABLE on first exec of a
            # freshly compiled NEFF; retry.
            import traceback as _tb

            _tb.print_exc()
            if attempt == 2:
                raise
            import time as _time

            _time.sleep(2.0)
    LAST_RESULTS = res

    out_full = np.empty((B, N_MOVES), dtype=np.float32)
    for i in range(N_CORES):
        fcols = placement[i]
        out_full[:, fcols] = res.results[i]["out"][: len(fcols)].T.astype(np.float32)
    return out_full
